# revision 33
# baseline (speedup 1.0000x reference)
"""MoE FFN (grouped top-1 routing, SwiGLU experts) on 8 Trainium2 NeuronCores.

Strategy (expert-parallel with quarter-FFN sharding for perfect balance):
  - Host computes the (tiny) routers: sigmoid(x @ macro_w) -> top-1 group of 4;
    within the selected group both 2 experts are active with
    sigmoid-normalized weights.
  - Tokens are sorted by routed group into one replicated array xs[D, W]
    (per-group segments at fixed padded offsets).
  - The 8 experts x 4 F-quarters = 32 weight shards are dealt so that every
    core gets exactly one shard of each GROUP (core c, group g -> expert
    2g + c//4, F-quarter c%4).  Every core therefore runs the identical
    amount of work on identically-shaped segments: perfect SPMD balance.
  - Device: for each group segment, Y_q^T = dwq^T @ (silu(gwq^T X^T) *
    (uwq^T X^T)) with features on partitions, tokens on the free dim, bf16
    in / fp32 PSUM / fp16 partial outputs.
  - Host combines: per token, y = w0 * sum(4 quarter partials of expert A)
    + w1 * sum(quarter partials of expert B), then unsorts.  The per-token
    router weights are applied host-side (linear in the down-projection),
    so no weighted copy of x needs to be shipped.

Scheduling notes (v2):
  - Boot = [x first-chunk | gate-fo0 | up-fo0] only (~1 MB) so the first
    real matmul fires ~2 us earlier; fo1..3 weight packs are separate DMAs
    that land during the first chunk's compute.
  - Outputs are staged per (group, chunk) as one [P, DO, c] SBUF tile and
    shipped as a single fully-contiguous block DMA (16 KB packets instead
    of 536 B ones); the last chunk of the last group is kept small so the
    trailing DMA after the final matmul is ~0.5 us.
"""


import ml_dtypes
import numpy as np

import concourse.bass as bass  # noqa: F401  (bass types via bacc)
import concourse.mybir as mybir
import concourse.tile as tile
from concourse import bacc
from concourse.bass_utils import run_bass_kernel_spmd

P = 128
D_MODEL = 1024
FFN_DIM = 2048
NUM_EXPERTS = 8
NUM_GROUPS = 4
FQ = FFN_DIM // 4  # F-quarter = 512
DO = D_MODEL // P  # 8 k-tiles over D
FO = FQ // P  # 4 f-tiles over an F-quarter
EPS = 1e-9

F32 = mybir.dt.float32
F16 = mybir.dt.float16
BF16 = mybir.dt.bfloat16

N_CORES = 8
N_WARM = 90  # dummy matmuls to lift the PE HAM throttle during DMA startup

_BUILD_CACHE: dict[tuple, object] = {}
LAST_RESULTS = None  # stashed BassKernelResults for test harnesses


def _build(caps: tuple[tuple[int, ...], ...]):
    """Bass/Tile program: 4 group segments, each one (expert, F-quarter) shard.

    caps: per group, the tuple of chunk sizes; segment capacity = sum.
    """
    Cs = [sum(ch) for ch in caps]
    offs = [sum(Cs[:g]) for g in range(NUM_GROUPS)]
    W = sum(Cs)
    c00 = caps[0][0]

    nc = bacc.Bacc(
        "TRN2",
        target_bir_lowering=False,
        debug=False,
        enable_asserts=False,
        num_devices=N_CORES,
    )
    # All inputs ship in partition-major layout matching the SBUF tiles
    # exactly: per-partition rows are fully contiguous -> max DMA bursts.
    # boot = first x chunk + fo0 gate/up weight tiles: the minimum needed to
    # start the first 16-matmul block.  The two pieces race on separate
    # queues (sync / vector) so the first matmul fires as early as possible.
    boot1 = nc.dram_tensor(
        "boot1", [P, DO, c00 + P], BF16, kind="ExternalInput"
    ).ap()
    bootu = nc.dram_tensor("bootu", [P, DO, P], BF16, kind="ExternalInput").ap()
    # fo1..3 weight packs for group 0, each [gate_fo | up_fo]; separate
    # transfers so fo1 can start as soon as its own pack lands.
    wfos = {
        f: nc.dram_tensor(f"wfo{f}", [P, DO, 2 * P], BF16, kind="ExternalInput").ap()
        for f in range(1, FO)
    }
    xrests = {}
    if Cs[0] > c00:
        xrests["xr0", c00, Cs[0]] = nc.dram_tensor(
            "xr0", [P, DO, Cs[0] - c00], BF16, kind="ExternalInput"
        ).ap()
    for g in range(1, NUM_GROUPS):
        xrests[f"xg{g}", offs[g], offs[g] + Cs[g]] = nc.dram_tensor(
            f"xg{g}", [P, DO, Cs[g]], BF16, kind="ExternalInput"
        ).ap()
    bgus = {
        g: nc.dram_tensor(f"bgu{g}", [P, DO, 2 * FQ], BF16, kind="ExternalInput").ap()
        for g in range(1, NUM_GROUPS)
    }
    bdws = [
        nc.dram_tensor(f"bdw{g}", [P, FO, D_MODEL], BF16, kind="ExternalInput").ap()
        for g in range(NUM_GROUPS)
    ]
    # per-(group, chunk) contiguous output blocks [P, DO, c] fp16
    ybs = {
        (g, cc): nc.dram_tensor(
            f"yb{g}_{cc}", [P, DO, c], F16, kind="ExternalOutput"
        ).ap()
        for g in range(NUM_GROUPS)
        for cc, c in enumerate(caps[g])
    }
    wy = nc.dram_tensor("wy", [P, 64], F32, kind="ExternalOutput").ap()

    with tile.TileContext(nc) as tc:
        with (
            tc.tile_pool(name="wu", bufs=1) as wup,
            tc.tile_pool(name="xp", bufs=1) as xp,
            tc.tile_pool(name="hp", bufs=1) as hp,
            tc.tile_pool(name="gp", bufs=1) as gp,
            tc.tile_pool(name="dp", bufs=1) as dp,
            tc.tile_pool(name="sp", bufs=4) as sp,
            tc.tile_pool(name="yp", bufs=2) as yp,
            tc.tile_pool(name="pg", bufs=2, space="PSUM") as pgp,
            tc.tile_pool(name="pu", bufs=2, space="PSUM") as pup,
            tc.tile_pool(name="pd", bufs=4, space="PSUM") as pdp,
        ):
            # boot DMAs first: [x chunk | gate_fo0] then [up_fo0] as two
            # sequential transfers on the sync HWDGE queue — the psg matmuls
            # only need the first piece, so they fire ~0.6us earlier.
            bt1 = xp.tile([P, DO, c00 + P], BF16, tag="bt1")
            nc.sync.dma_start(bt1[:], boot1[:])
            btu = xp.tile([P, DO, P], BF16, tag="btu")
            nc.sync.dma_start(btu[:], bootu[:])

            # ── PE warm-up: dense dummy matmuls while the startup DMAs fly ──
            wt = wup.tile([P, P], BF16, tag="wt")
            nc.vector.memset(wt[:], 0.0)
            pws = pgp.tile([P, 64], F32, tag="psg", name="psg_warm")
            for i in range(N_WARM):
                nc.tensor.matmul(
                    pws[:], wt[:], wt[:, 0:64],
                    start=(i == 0), stop=(i == N_WARM - 1),
                )

            # ── remaining input DMAs on ONE queue (sync), in need-order ─────
            # HBM bandwidth (~440 GB/s/core) is shared across queues, so a
            # second concurrent input stream only delays the critical path.
            wfts = {}
            for f in range(1, FO):
                wfts[f] = gp.tile([P, DO, 2 * P], BF16, tag=f"wf{f}", name=f"wf_{f}")
                nc.sync.dma_start(wfts[f][:], wfos[f][:])
            xss = xp.tile([P, DO, W], BF16, tag="xs")
            for (nm, a, b), src in xrests.items():
                if nm != "xr0":
                    continue
                nc.sync.dma_start(xss[:, :, a:b], src[:])
            guts = {}
            for g in range(1, NUM_GROUPS):
                guts[g] = gp.tile(
                    [P, DO, 2 * FQ], BF16, tag=f"gut{g}", name=f"gut_{g}"
                )
                nc.sync.dma_start(guts[g][:], bgus[g][:])
                for (nm, a, b), src in xrests.items():
                    if nm != f"xg{g}":
                        continue
                    nc.sync.dma_start(xss[:, :, a:b], src[:])
            dts = {}
            for g in range(NUM_GROUPS):
                dts[g] = dp.tile([P, FO, D_MODEL], BF16, tag=f"dt{g}", name=f"dt_{g}")
                nc.sync.dma_start(dts[g][:], bdws[g][:])

            # keep the warm-up matmuls from being dead-code-eliminated
            wys = wup.tile([P, 64], F32, tag="wy")
            nc.vector.tensor_copy(out=wys[:], in_=pws[:])
            nc.sync.dma_start(wy[:, :], wys[:])

            # ── phase 1: gate/up + SwiGLU for all 4 group segments ──────────
            hs = {}
            for g in range(NUM_GROUPS):
                hs[g] = hp.tile([P, FO, Cs[g]], BF16, tag=f"h{g}", name=f"h{g}")
                coff = 0
                for cc, chunk in enumerate(caps[g]):
                    cs = slice(coff, coff + chunk)
                    xcs = slice(offs[g] + coff, offs[g] + coff + chunk)
                    for fo in range(FO):
                        if g == 0:
                            if fo == 0:
                                gt_, go_, ut_, uo_ = bt1, c00, btu, 0
                            else:
                                gt_, go_, ut_, uo_ = wfts[fo], 0, wfts[fo], P
                        else:
                            gt_, go_ = guts[g], fo * P
                            ut_, uo_ = guts[g], FQ + fo * P
                        psg = pgp.tile([P, chunk], F32, tag="psg",
                                       name=f"psg_{g}_{cc}_{fo}")
                        psu = pup.tile([P, chunk], F32, tag="psu",
                                       name=f"psu_{g}_{cc}_{fo}")
                        boot = g == 0 and cc == 0
                        for do in range(DO):
                            nc.tensor.matmul(
                                psg[:], gt_[:, do, go_ : go_ + P],
                                bt1[:, do, 0:c00] if boot else xss[:, do, xcs],
                                start=(do == 0), stop=(do == DO - 1),
                            )
                        for do in range(DO):
                            nc.tensor.matmul(
                                psu[:], ut_[:, do, uo_ : uo_ + P],
                                bt1[:, do, 0:c00] if boot else xss[:, do, xcs],
                                start=(do == 0), stop=(do == DO - 1),
                            )
                        sg = sp.tile([P, chunk], F32, tag="sg")
                        nc.scalar.activation(
                            sg[:], psg[:], mybir.ActivationFunctionType.Silu
                        )
                        nc.vector.tensor_mul(
                            out=hs[g][:, fo, cs], in0=sg[:], in1=psu[:]
                        )
                    coff += chunk

            # ── phase 2: down-projection for all 4 group segments ───────────
            # outputs staged per (g, cc) in one [P, DO, c] tile, shipped as a
            # single fully-contiguous DMA -> large packets, tiny tail.
            nq = 0
            for g in range(NUM_GROUPS):
                coff = 0
                for cc, chunk in enumerate(caps[g]):
                    cs = slice(coff, coff + chunk)
                    yo = yp.tile([P, DO, chunk], F16, tag="yo",
                                 name=f"yo_{g}_{cc}")
                    for do in range(DO):
                        psy = pdp.tile([P, chunk], F32, tag="psy",
                                       name=f"psy_{g}_{cc}_{do}")
                        for fo in range(FO):
                            nc.tensor.matmul(
                                psy[:],
                                dts[g][:, fo, do * P : (do + 1) * P],
                                hs[g][:, fo, cs],
                                start=(fo == 0), stop=(fo == FO - 1),
                            )
                        if nq % 2 == 0:
                            nc.scalar.activation(
                                yo[:, do, :], psy[:],
                                mybir.ActivationFunctionType.Copy,
                            )
                        else:
                            nc.vector.tensor_copy(out=yo[:, do, :], in_=psy[:])
                        nq += 1
                        # ship each half as soon as its 4 do-slices are
                        # copied: the trailing DMA after the last matmul is
                        # only half a block (~0.3 MB) instead of 1+ MB.  The
                        # very last block ships in quarters so the final
                        # piece is ~0.14 MB.
                        last_blk = (g == NUM_GROUPS - 1
                                    and cc == len(caps[g]) - 1)
                        if do == 3:
                            nc.sync.dma_start(
                                ybs[g, cc][:, 0:4, :], yo[:, 0:4, :]
                            )
                        elif last_blk and do == 5:
                            nc.sync.dma_start(
                                ybs[g, cc][:, 4:6, :], yo[:, 4:6, :]
                            )
                        elif last_blk and do == DO - 1:
                            nc.sync.dma_start(
                                ybs[g, cc][:, 6:DO, :], yo[:, 6:DO, :]
                            )
                        elif do == DO - 1:
                            nc.sync.dma_start(
                                ybs[g, cc][:, 4:DO, :], yo[:, 4:DO, :]
                            )
                    coff += chunk
    nc.finalize()
    return nc


def _get_program(caps):
    if caps not in _BUILD_CACHE:
        _BUILD_CACHE[caps] = _build(caps)
    return _BUILD_CACHE[caps]


def _sigmoid(z):
    return 1.0 / (1.0 + np.exp(-z))


def _route(xf32, macro_w, micro_w):
    """Host routers in float64. Returns group index per token and per-token
    weights for the 2 experts of the selected group (float32)."""
    xf = xf32.astype(np.float64)
    ms = _sigmoid(xf @ macro_w.astype(np.float64))  # [T, G]
    g_sel = np.argmax(ms, axis=1)
    T = xf.shape[0]
    mval = ms[np.arange(T), g_sel]
    mv = mval / (mval + EPS)

    w2 = np.zeros((T, 2), np.float64)
    for g in range(NUM_GROUPS):
        idx = np.nonzero(g_sel == g)[0]
        if idx.size == 0:
            continue
        s = _sigmoid(xf[idx] @ micro_w[g].astype(np.float64))  # [n, 2]
        denom = np.maximum(s[:, 0], s[:, 1]) + np.minimum(s[:, 0], s[:, 1]) + EPS
        w2[idx, 0] = mv[idx] * s[:, 0] / denom
        w2[idx, 1] = mv[idx] * s[:, 1] / denom
    return g_sel, w2.astype(np.float32)


def _pad4(v: int) -> int:
    return -(-v // 4) * 4


def _even_chunks(n: int, cmax: int = 512) -> list[int]:
    """Even split of n into ceil(n/cmax) chunks, each padded to %4."""
    n = max(n, 8)
    k = -(-n // cmax)
    base = -(-n // k)
    out = [_pad4(base)] * (k - 1)
    out.append(_pad4(n - base * (k - 1)))
    return out


def _plan_chunks(counts: list[int]) -> tuple[tuple[int, ...], ...]:
    """Chunk plan per group: small boot chunk for group 0 (fast first matmul),
    small final chunk for the last group (tiny trailing output DMA)."""
    plans = []
    for g, n in enumerate(counts):
        n = max(n, 8)
        if g == 0:
            if n <= 288:
                plans.append((_pad4(n),))
            else:
                plans.append((256, *_even_chunks(n - 256)))
        else:
            plans.append(tuple(_even_chunks(n)))
    return tuple(plans)


def kernel(x, macro_w, micro_w, gate_w, up_w, down_w):
    global LAST_RESULTS
    x = np.asarray(x)
    B, S, D = x.shape
    T = B * S
    xf = np.ascontiguousarray(x.reshape(T, D).astype(np.float32, copy=False))

    g_sel, w2 = _route(xf, np.asarray(macro_w), np.asarray(micro_w))
    idx_by_g = [np.nonzero(g_sel == g)[0] for g in range(NUM_GROUPS)]

    caps = _plan_chunks([ix.size for ix in idx_by_g])
    Cs = [sum(ch) for ch in caps]
    offs = [sum(Cs[:g]) for g in range(NUM_GROUPS)]
    W = sum(Cs)
    c00 = caps[0][0]
    nc = _get_program(caps)

    # group-sorted, padded token matrix [D, W] bf16 (replicated to all cores)
    xs = np.zeros((D, W), ml_dtypes.bfloat16)
    for g in range(NUM_GROUPS):
        ix = idx_by_g[g]
        if ix.size:
            xs[:, offs[g] : offs[g] + ix.size] = xf[ix].T.astype(ml_dtypes.bfloat16)

    # bf16 weights in partition-major [p, do/fo, f/d] layout (contiguous DMA)
    gate_b = np.asarray(gate_w, np.float32).astype(ml_dtypes.bfloat16)
    up_b = np.asarray(up_w, np.float32).astype(ml_dtypes.bfloat16)
    down_b = np.asarray(down_w, np.float32).astype(ml_dtypes.bfloat16)
    # [E, D, F] -> [E, DO, P, F] -> [E, P, DO, F]
    gate_p = gate_b.reshape(NUM_EXPERTS, DO, P, FFN_DIM).transpose(0, 2, 1, 3)
    up_p = up_b.reshape(NUM_EXPERTS, DO, P, FFN_DIM).transpose(0, 2, 1, 3)
    # [E, F, D] -> [E, 4, FO, P, D] -> [E, 4, P, FO, D]
    down_p = down_b.reshape(NUM_EXPERTS, 4, FO, P, D_MODEL).transpose(0, 1, 3, 2, 4)

    # partition-major token array [p, do, c]; ship as contiguous blocks
    xsp = xs.reshape(DO, P, W).transpose(1, 0, 2)
    xboot = np.ascontiguousarray(xsp[:, :, 0:c00])
    xparts = {}
    if Cs[0] > c00:
        xparts["xr0"] = np.ascontiguousarray(xsp[:, :, c00 : Cs[0]])
    for g in range(1, NUM_GROUPS):
        xparts[f"xg{g}"] = np.ascontiguousarray(
            xsp[:, :, offs[g] : offs[g] + Cs[g]]
        )

    in_maps = []
    for c in range(N_CORES):
        m = dict(xparts)
        b = c // 4  # which expert of each group
        q = c % 4  # which F-quarter
        fsl = slice(q * FQ, (q + 1) * FQ)
        for g in range(NUM_GROUPS):
            e = 2 * g + b
            if g == 0:
                gq = gate_p[e][:, :, fsl]
                uq = up_p[e][:, :, fsl]
                m["boot1"] = np.ascontiguousarray(
                    np.concatenate([xboot, gq[:, :, 0:P]], axis=2)
                )
                m["bootu"] = np.ascontiguousarray(uq[:, :, 0:P])
                for f in range(1, FO):
                    m[f"wfo{f}"] = np.ascontiguousarray(
                        np.concatenate(
                            [
                                gq[:, :, f * P : (f + 1) * P],
                                uq[:, :, f * P : (f + 1) * P],
                            ],
                            axis=2,
                        )
                    )
            else:
                bgu = np.empty((P, DO, 2 * FQ), ml_dtypes.bfloat16)
                bgu[:, :, :FQ] = gate_p[e][:, :, fsl]
                bgu[:, :, FQ:] = up_p[e][:, :, fsl]
                m[f"bgu{g}"] = bgu
            m[f"bdw{g}"] = np.ascontiguousarray(down_p[e, q])
        in_maps.append(m)

    res = run_bass_kernel_spmd(nc, in_maps, core_ids=list(range(N_CORES)))
    LAST_RESULTS = res

    y = np.zeros((T, D), np.float32)
    for g in range(NUM_GROUPS):
        ix = idx_by_g[g]
        if ix.size == 0:
            continue
        # stitch the per-chunk blocks into [D, C_g] partials per expert half
        pa = np.zeros((D, Cs[g]), np.float32)
        pb = np.zeros((D, Cs[g]), np.float32)
        coff = 0
        for cc, chunk in enumerate(caps[g]):
            for c in range(N_CORES):
                blk = res.results[c][f"yb{g}_{cc}"].astype(np.float32)
                # blk[p, do, j] = y_d[do*128+p] for token j
                part = blk.transpose(1, 0, 2).reshape(D, chunk)
                if c < 4:
                    pa[:, coff : coff + chunk] += part
                else:
                    pb[:, coff : coff + chunk] += part
            coff += chunk
        seg = slice(0, ix.size)
        y[ix] = pa[:, seg].T * w2[ix, 0:1] + pb[:, seg].T * w2[ix, 1:2]
    return y.reshape(B, S, D)


# revision 38
# speedup vs baseline: 1.0068x; 1.0068x over previous
"""MoE FFN (grouped top-1 routing, SwiGLU experts) on 8 Trainium2 NeuronCores.

Strategy (expert-parallel with quarter-FFN sharding for perfect balance):
  - Host computes the (tiny) routers: sigmoid(x @ macro_w) -> top-1 group of 4;
    within the selected group both 2 experts are active with
    sigmoid-normalized weights.
  - Tokens are sorted by routed group into one replicated array xs[D, W]
    (per-group segments at fixed padded offsets).
  - The 8 experts x 4 F-quarters = 32 weight shards are dealt so that every
    core gets exactly one shard of each GROUP (core c, group g -> expert
    2g + c//4, F-quarter c%4).  Every core therefore runs the identical
    amount of work on identically-shaped segments: perfect SPMD balance.
  - Device: for each group segment, Y_q^T = dwq^T @ (silu(gwq^T X^T) *
    (uwq^T X^T)) with features on partitions, tokens on the free dim, bf16
    in / fp32 PSUM / fp16 partial outputs.
  - Host combines: per token, y = w0 * sum(4 quarter partials of expert A)
    + w1 * sum(quarter partials of expert B), then unsorts.  The per-token
    router weights are applied host-side (linear in the down-projection),
    so no weighted copy of x needs to be shipped.

Scheduling notes (v2):
  - Boot = [x first-chunk | gate-fo0 | up-fo0] only (~1 MB) so the first
    real matmul fires ~2 us earlier; fo1..3 weight packs are separate DMAs
    that land during the first chunk's compute.
  - Outputs are staged per (group, chunk) as one [P, DO, c] SBUF tile and
    shipped as a single fully-contiguous block DMA (16 KB packets instead
    of 536 B ones); the last chunk of the last group is kept small so the
    trailing DMA after the final matmul is ~0.5 us.
"""


import ml_dtypes
import numpy as np

import concourse.bass as bass  # noqa: F401  (bass types via bacc)
import concourse.mybir as mybir
import concourse.tile as tile
from concourse import bacc
from concourse.bass_utils import run_bass_kernel_spmd

P = 128
D_MODEL = 1024
FFN_DIM = 2048
NUM_EXPERTS = 8
NUM_GROUPS = 4
FQ = FFN_DIM // 4  # F-quarter = 512
DO = D_MODEL // P  # 8 k-tiles over D
FO = FQ // P  # 4 f-tiles over an F-quarter
EPS = 1e-9

F32 = mybir.dt.float32
F16 = mybir.dt.float16
BF16 = mybir.dt.bfloat16

N_CORES = 8
N_WARM = 90  # dummy matmuls to lift the PE HAM throttle during DMA startup

_BUILD_CACHE: dict[tuple, object] = {}
LAST_RESULTS = None  # stashed BassKernelResults for test harnesses


def _build(caps: tuple[tuple[int, ...], ...]):
    """Bass/Tile program: 4 group segments, each one (expert, F-quarter) shard.

    caps: per group, the tuple of chunk sizes; segment capacity = sum.
    """
    Cs = [sum(ch) for ch in caps]
    offs = [sum(Cs[:g]) for g in range(NUM_GROUPS)]
    W = sum(Cs)
    c00 = caps[0][0]

    nc = bacc.Bacc(
        "TRN2",
        target_bir_lowering=False,
        debug=False,
        enable_asserts=False,
        num_devices=N_CORES,
    )
    # All inputs ship in partition-major layout matching the SBUF tiles
    # exactly: per-partition rows are fully contiguous -> max DMA bursts.
    # boot = first x chunk + fo0 gate/up weight tiles: the minimum needed to
    # start the first 16-matmul block.  The two pieces race on separate
    # queues (sync / vector) so the first matmul fires as early as possible.
    boot1 = nc.dram_tensor(
        "boot1", [P, DO, c00 + 2 * P], BF16, kind="ExternalInput"
    ).ap()
    # fo1..3 weight packs for group 0, each [gate_fo | up_fo]; separate
    # transfers so fo1 can start as soon as its own pack lands.
    wfos = {
        f: nc.dram_tensor(f"wfo{f}", [P, DO, 2 * P], BF16, kind="ExternalInput").ap()
        for f in range(1, FO)
    }
    xrests = {}
    if Cs[0] > c00:
        xrests["xr0", c00, Cs[0]] = nc.dram_tensor(
            "xr0", [P, DO, Cs[0] - c00], BF16, kind="ExternalInput"
        ).ap()
    for g in range(1, NUM_GROUPS):
        xrests[f"xg{g}", offs[g], offs[g] + Cs[g]] = nc.dram_tensor(
            f"xg{g}", [P, DO, Cs[g]], BF16, kind="ExternalInput"
        ).ap()
    bgus = {
        g: nc.dram_tensor(f"bgu{g}", [P, DO, 2 * FQ], BF16, kind="ExternalInput").ap()
        for g in range(1, NUM_GROUPS)
    }
    bdws = [
        nc.dram_tensor(f"bdw{g}", [P, FO, D_MODEL], BF16, kind="ExternalInput").ap()
        for g in range(NUM_GROUPS)
    ]
    # per-(group, chunk) contiguous output blocks [P, DO, c] fp16
    ybs = {
        (g, cc): nc.dram_tensor(
            f"yb{g}_{cc}", [P, DO, c], F16, kind="ExternalOutput"
        ).ap()
        for g in range(NUM_GROUPS)
        for cc, c in enumerate(caps[g])
    }
    wy = nc.dram_tensor("wy", [P, 64], F32, kind="ExternalOutput").ap()

    with tile.TileContext(nc) as tc:
        with (
            tc.tile_pool(name="wu", bufs=1) as wup,
            tc.tile_pool(name="xp", bufs=1) as xp,
            tc.tile_pool(name="hp", bufs=1) as hp,
            tc.tile_pool(name="gp", bufs=1) as gp,
            tc.tile_pool(name="dp", bufs=1) as dp,
            tc.tile_pool(name="sp", bufs=4) as sp,
            tc.tile_pool(name="yp", bufs=2) as yp,
            tc.tile_pool(name="pg", bufs=3, space="PSUM") as pgp,
            tc.tile_pool(name="pu", bufs=2, space="PSUM") as pup,
            tc.tile_pool(name="pd", bufs=3, space="PSUM") as pdp,
        ):
            # boot DMA first: first x chunk + fo0 gate/up weights in one
            # contiguous transfer on the sync HWDGE queue.
            bt1 = xp.tile([P, DO, c00 + 2 * P], BF16, tag="bt1")
            nc.sync.dma_start(bt1[:], boot1[:])

            # ── PE warm-up: dense dummy matmuls while the startup DMAs fly ──
            wt = wup.tile([P, P], BF16, tag="wt")
            nc.vector.memset(wt[:], 0.0)
            pws = pgp.tile([P, 64], F32, tag="psg", name="psg_warm")
            for i in range(N_WARM):
                nc.tensor.matmul(
                    pws[:], wt[:], wt[:, 0:64],
                    start=(i == 0), stop=(i == N_WARM - 1),
                )

            # ── remaining input DMAs on ONE queue (sync), in need-order ─────
            # HBM bandwidth (~440 GB/s/core) is shared across queues, so a
            # second concurrent input stream only delays the critical path.
            wfts = {}
            for f in range(1, FO):
                wfts[f] = gp.tile([P, DO, 2 * P], BF16, tag=f"wf{f}", name=f"wf_{f}")
                nc.sync.dma_start(wfts[f][:], wfos[f][:])
            xss = xp.tile([P, DO, W], BF16, tag="xs")
            for (nm, a, b), src in xrests.items():
                if nm != "xr0":
                    continue
                nc.sync.dma_start(xss[:, :, a:b], src[:])
            guts = {}
            for g in range(1, NUM_GROUPS):
                guts[g] = gp.tile(
                    [P, DO, 2 * FQ], BF16, tag=f"gut{g}", name=f"gut_{g}"
                )
                nc.sync.dma_start(guts[g][:], bgus[g][:])
                for (nm, a, b), src in xrests.items():
                    if nm != f"xg{g}":
                        continue
                    nc.sync.dma_start(xss[:, :, a:b], src[:])
            dts = {}
            for g in range(NUM_GROUPS):
                dts[g] = dp.tile([P, FO, D_MODEL], BF16, tag=f"dt{g}", name=f"dt_{g}")
                nc.sync.dma_start(dts[g][:], bdws[g][:])

            # keep the warm-up matmuls from being dead-code-eliminated
            wys = wup.tile([P, 64], F32, tag="wy")
            nc.vector.tensor_copy(out=wys[:], in_=pws[:])
            nc.sync.dma_start(wy[:, :], wys[:])

            # ── phase 1: gate/up + SwiGLU for all 4 group segments ──────────
            hs = {}
            for g in range(NUM_GROUPS):
                hs[g] = hp.tile([P, FO, Cs[g]], BF16, tag=f"h{g}", name=f"h{g}")
                coff = 0
                for cc, chunk in enumerate(caps[g]):
                    cs = slice(coff, coff + chunk)
                    xcs = slice(offs[g] + coff, offs[g] + coff + chunk)
                    for fo in range(FO):
                        if g == 0:
                            if fo == 0:
                                gt_, go_, ut_, uo_ = bt1, c00, bt1, c00 + P
                            else:
                                gt_, go_, ut_, uo_ = wfts[fo], 0, wfts[fo], P
                        else:
                            gt_, go_ = guts[g], fo * P
                            ut_, uo_ = guts[g], FQ + fo * P
                        psg = pgp.tile([P, chunk], F32, tag="psg",
                                       name=f"psg_{g}_{cc}_{fo}")
                        psu = pup.tile([P, chunk], F32, tag="psu",
                                       name=f"psu_{g}_{cc}_{fo}")
                        boot = g == 0 and cc == 0
                        for do in range(DO):
                            nc.tensor.matmul(
                                psg[:], gt_[:, do, go_ : go_ + P],
                                bt1[:, do, 0:c00] if boot else xss[:, do, xcs],
                                start=(do == 0), stop=(do == DO - 1),
                            )
                        for do in range(DO):
                            nc.tensor.matmul(
                                psu[:], ut_[:, do, uo_ : uo_ + P],
                                bt1[:, do, 0:c00] if boot else xss[:, do, xcs],
                                start=(do == 0), stop=(do == DO - 1),
                            )
                        sg = sp.tile([P, chunk], F32, tag="sg")
                        nc.scalar.activation(
                            sg[:], psg[:], mybir.ActivationFunctionType.Silu
                        )
                        nc.vector.tensor_mul(
                            out=hs[g][:, fo, cs], in0=sg[:], in1=psu[:]
                        )
                    coff += chunk

            # ── phase 2: down-projection for all 4 group segments ───────────
            # outputs staged per (g, cc) in one [P, DO, c] tile, shipped as a
            # single fully-contiguous DMA -> large packets, tiny tail.
            nq = 0
            for g in range(NUM_GROUPS):
                coff = 0
                for cc, chunk in enumerate(caps[g]):
                    cs = slice(coff, coff + chunk)
                    yo = yp.tile([P, DO, chunk], F16, tag="yo",
                                 name=f"yo_{g}_{cc}")
                    for do in range(DO):
                        psy = pdp.tile([P, chunk], F32, tag="psy",
                                       name=f"psy_{g}_{cc}_{do}")
                        for fo in range(FO):
                            nc.tensor.matmul(
                                psy[:],
                                dts[g][:, fo, do * P : (do + 1) * P],
                                hs[g][:, fo, cs],
                                start=(fo == 0), stop=(fo == FO - 1),
                            )
                        if nq % 2 == 0:
                            nc.scalar.activation(
                                yo[:, do, :], psy[:],
                                mybir.ActivationFunctionType.Copy,
                            )
                        else:
                            nc.vector.tensor_copy(out=yo[:, do, :], in_=psy[:])
                        nq += 1
                        # ship each half as soon as its 4 do-slices are
                        # copied: the trailing DMA after the last matmul is
                        # only half a block (~0.3 MB) instead of 1+ MB.  The
                        # very last block ships in quarters so the final
                        # piece is ~0.14 MB.
                        last_blk = (g == NUM_GROUPS - 1
                                    and cc == len(caps[g]) - 1)
                        if do == 3:
                            nc.sync.dma_start(
                                ybs[g, cc][:, 0:4, :], yo[:, 0:4, :]
                            )
                        elif last_blk and do == 5:
                            nc.sync.dma_start(
                                ybs[g, cc][:, 4:6, :], yo[:, 4:6, :]
                            )
                        elif last_blk and do == DO - 1:
                            nc.sync.dma_start(
                                ybs[g, cc][:, 6:DO, :], yo[:, 6:DO, :]
                            )
                        elif do == DO - 1:
                            nc.sync.dma_start(
                                ybs[g, cc][:, 4:DO, :], yo[:, 4:DO, :]
                            )
                    coff += chunk
    nc.finalize()
    return nc


def _get_program(caps):
    if caps not in _BUILD_CACHE:
        _BUILD_CACHE[caps] = _build(caps)
    return _BUILD_CACHE[caps]


def _sigmoid(z):
    return 1.0 / (1.0 + np.exp(-z))


def _route(xf32, macro_w, micro_w):
    """Host routers in float64. Returns group index per token and per-token
    weights for the 2 experts of the selected group (float32)."""
    xf = xf32.astype(np.float64)
    ms = _sigmoid(xf @ macro_w.astype(np.float64))  # [T, G]
    g_sel = np.argmax(ms, axis=1)
    T = xf.shape[0]
    mval = ms[np.arange(T), g_sel]
    mv = mval / (mval + EPS)

    w2 = np.zeros((T, 2), np.float64)
    for g in range(NUM_GROUPS):
        idx = np.nonzero(g_sel == g)[0]
        if idx.size == 0:
            continue
        s = _sigmoid(xf[idx] @ micro_w[g].astype(np.float64))  # [n, 2]
        denom = np.maximum(s[:, 0], s[:, 1]) + np.minimum(s[:, 0], s[:, 1]) + EPS
        w2[idx, 0] = mv[idx] * s[:, 0] / denom
        w2[idx, 1] = mv[idx] * s[:, 1] / denom
    return g_sel, w2.astype(np.float32)


def _pad4(v: int) -> int:
    return -(-v // 4) * 4


def _even_chunks(n: int, cmax: int = 512) -> list[int]:
    """Even split of n into ceil(n/cmax) chunks, each padded to %4."""
    n = max(n, 8)
    k = -(-n // cmax)
    base = -(-n // k)
    out = [_pad4(base)] * (k - 1)
    out.append(_pad4(n - base * (k - 1)))
    return out


def _plan_chunks(counts: list[int]) -> tuple[tuple[int, ...], ...]:
    """Chunk plan per group: small boot chunk for group 0 (fast first matmul),
    small final chunk for the last group (tiny trailing output DMA)."""
    plans = []
    for g, n in enumerate(counts):
        n = max(n, 8)
        if g == 0:
            if n <= 288:
                plans.append((_pad4(n),))
            else:
                plans.append((256, *_even_chunks(n - 256)))
        else:
            plans.append(tuple(_even_chunks(n)))
    return tuple(plans)


def kernel(x, macro_w, micro_w, gate_w, up_w, down_w):
    global LAST_RESULTS
    x = np.asarray(x)
    B, S, D = x.shape
    T = B * S
    xf = np.ascontiguousarray(x.reshape(T, D).astype(np.float32, copy=False))

    g_sel, w2 = _route(xf, np.asarray(macro_w), np.asarray(micro_w))
    idx_by_g = [np.nonzero(g_sel == g)[0] for g in range(NUM_GROUPS)]

    caps = _plan_chunks([ix.size for ix in idx_by_g])
    Cs = [sum(ch) for ch in caps]
    offs = [sum(Cs[:g]) for g in range(NUM_GROUPS)]
    W = sum(Cs)
    c00 = caps[0][0]
    nc = _get_program(caps)

    # group-sorted, padded token matrix [D, W] bf16 (replicated to all cores)
    xs = np.zeros((D, W), ml_dtypes.bfloat16)
    for g in range(NUM_GROUPS):
        ix = idx_by_g[g]
        if ix.size:
            xs[:, offs[g] : offs[g] + ix.size] = xf[ix].T.astype(ml_dtypes.bfloat16)

    # bf16 weights in partition-major [p, do/fo, f/d] layout (contiguous DMA)
    gate_b = np.asarray(gate_w, np.float32).astype(ml_dtypes.bfloat16)
    up_b = np.asarray(up_w, np.float32).astype(ml_dtypes.bfloat16)
    down_b = np.asarray(down_w, np.float32).astype(ml_dtypes.bfloat16)
    # [E, D, F] -> [E, DO, P, F] -> [E, P, DO, F]
    gate_p = gate_b.reshape(NUM_EXPERTS, DO, P, FFN_DIM).transpose(0, 2, 1, 3)
    up_p = up_b.reshape(NUM_EXPERTS, DO, P, FFN_DIM).transpose(0, 2, 1, 3)
    # [E, F, D] -> [E, 4, FO, P, D] -> [E, 4, P, FO, D]
    down_p = down_b.reshape(NUM_EXPERTS, 4, FO, P, D_MODEL).transpose(0, 1, 3, 2, 4)

    # partition-major token array [p, do, c]; ship as contiguous blocks
    xsp = xs.reshape(DO, P, W).transpose(1, 0, 2)
    xboot = np.ascontiguousarray(xsp[:, :, 0:c00])
    xparts = {}
    if Cs[0] > c00:
        xparts["xr0"] = np.ascontiguousarray(xsp[:, :, c00 : Cs[0]])
    for g in range(1, NUM_GROUPS):
        xparts[f"xg{g}"] = np.ascontiguousarray(
            xsp[:, :, offs[g] : offs[g] + Cs[g]]
        )

    in_maps = []
    for c in range(N_CORES):
        m = dict(xparts)
        b = c // 4  # which expert of each group
        q = c % 4  # which F-quarter
        fsl = slice(q * FQ, (q + 1) * FQ)
        for g in range(NUM_GROUPS):
            e = 2 * g + b
            if g == 0:
                gq = gate_p[e][:, :, fsl]
                uq = up_p[e][:, :, fsl]
                m["boot1"] = np.ascontiguousarray(
                    np.concatenate(
                        [xboot, gq[:, :, 0:P], uq[:, :, 0:P]], axis=2
                    )
                )
                for f in range(1, FO):
                    m[f"wfo{f}"] = np.ascontiguousarray(
                        np.concatenate(
                            [
                                gq[:, :, f * P : (f + 1) * P],
                                uq[:, :, f * P : (f + 1) * P],
                            ],
                            axis=2,
                        )
                    )
            else:
                bgu = np.empty((P, DO, 2 * FQ), ml_dtypes.bfloat16)
                bgu[:, :, :FQ] = gate_p[e][:, :, fsl]
                bgu[:, :, FQ:] = up_p[e][:, :, fsl]
                m[f"bgu{g}"] = bgu
            m[f"bdw{g}"] = np.ascontiguousarray(down_p[e, q])
        in_maps.append(m)

    res = run_bass_kernel_spmd(nc, in_maps, core_ids=list(range(N_CORES)))
    LAST_RESULTS = res

    y = np.zeros((T, D), np.float32)
    for g in range(NUM_GROUPS):
        ix = idx_by_g[g]
        if ix.size == 0:
            continue
        # stitch the per-chunk blocks into [D, C_g] partials per expert half
        pa = np.zeros((D, Cs[g]), np.float32)
        pb = np.zeros((D, Cs[g]), np.float32)
        coff = 0
        for cc, chunk in enumerate(caps[g]):
            for c in range(N_CORES):
                blk = res.results[c][f"yb{g}_{cc}"].astype(np.float32)
                # blk[p, do, j] = y_d[do*128+p] for token j
                part = blk.transpose(1, 0, 2).reshape(D, chunk)
                if c < 4:
                    pa[:, coff : coff + chunk] += part
                else:
                    pb[:, coff : coff + chunk] += part
            coff += chunk
        seg = slice(0, ix.size)
        y[ix] = pa[:, seg].T * w2[ix, 0:1] + pb[:, seg].T * w2[ix, 1:2]
    return y.reshape(B, S, D)


# revision 39
# speedup vs baseline: 1.0098x; 1.0030x over previous
"""MoE FFN (grouped top-1 routing, SwiGLU experts) on 8 Trainium2 NeuronCores.

Strategy (expert-parallel with quarter-FFN sharding for perfect balance):
  - Host computes the (tiny) routers: sigmoid(x @ macro_w) -> top-1 group of 4;
    within the selected group both 2 experts are active with
    sigmoid-normalized weights.
  - Tokens are sorted by routed group into one replicated array xs[D, W]
    (per-group segments at fixed padded offsets).
  - The 8 experts x 4 F-quarters = 32 weight shards are dealt so that every
    core gets exactly one shard of each GROUP (core c, group g -> expert
    2g + c//4, F-quarter c%4).  Every core therefore runs the identical
    amount of work on identically-shaped segments: perfect SPMD balance.
  - Device: for each group segment, Y_q^T = dwq^T @ (silu(gwq^T X^T) *
    (uwq^T X^T)) with features on partitions, tokens on the free dim, bf16
    in / fp32 PSUM / fp16 partial outputs.
  - Host combines: per token, y = w0 * sum(4 quarter partials of expert A)
    + w1 * sum(quarter partials of expert B), then unsorts.  The per-token
    router weights are applied host-side (linear in the down-projection),
    so no weighted copy of x needs to be shipped.

Scheduling notes (v2):
  - Boot = [x first-chunk | gate-fo0 | up-fo0] only (~1 MB) so the first
    real matmul fires ~2 us earlier; fo1..3 weight packs are separate DMAs
    that land during the first chunk's compute.
  - Outputs are staged per (group, chunk) as one [P, DO, c] SBUF tile and
    shipped as a single fully-contiguous block DMA (16 KB packets instead
    of 536 B ones); the last chunk of the last group is kept small so the
    trailing DMA after the final matmul is ~0.5 us.
"""


import ml_dtypes
import numpy as np

import concourse.bass as bass  # noqa: F401  (bass types via bacc)
import concourse.mybir as mybir
import concourse.tile as tile
from concourse import bacc
from concourse.bass_utils import run_bass_kernel_spmd

P = 128
D_MODEL = 1024
FFN_DIM = 2048
NUM_EXPERTS = 8
NUM_GROUPS = 4
FQ = FFN_DIM // 4  # F-quarter = 512
DO = D_MODEL // P  # 8 k-tiles over D
FO = FQ // P  # 4 f-tiles over an F-quarter
EPS = 1e-9

F32 = mybir.dt.float32
F16 = mybir.dt.float16
BF16 = mybir.dt.bfloat16

N_CORES = 8
N_WARM = 90  # dummy matmuls to lift the PE HAM throttle during DMA startup

_BUILD_CACHE: dict[tuple, object] = {}
LAST_RESULTS = None  # stashed BassKernelResults for test harnesses


def _build(caps: tuple[tuple[int, ...], ...]):
    """Bass/Tile program: 4 group segments, each one (expert, F-quarter) shard.

    caps: per group, the tuple of chunk sizes; segment capacity = sum.
    """
    Cs = [sum(ch) for ch in caps]
    offs = [sum(Cs[:g]) for g in range(NUM_GROUPS)]
    W = sum(Cs)
    c00 = caps[0][0]

    nc = bacc.Bacc(
        "TRN2",
        target_bir_lowering=False,
        debug=False,
        enable_asserts=False,
        num_devices=N_CORES,
    )
    # All inputs ship in partition-major layout matching the SBUF tiles
    # exactly: per-partition rows are fully contiguous -> max DMA bursts.
    # boot = first x chunk + fo0 gate/up weight tiles: the minimum needed to
    # start the first 16-matmul block.  The two pieces race on separate
    # queues (sync / vector) so the first matmul fires as early as possible.
    boot1 = nc.dram_tensor(
        "boot1", [P, DO, c00 + 2 * P], BF16, kind="ExternalInput"
    ).ap()
    # fo1..3 weight packs for group 0, each [gate_fo | up_fo]; separate
    # transfers so fo1 can start as soon as its own pack lands.
    wfos = {
        f: nc.dram_tensor(f"wfo{f}", [P, DO, 2 * P], BF16, kind="ExternalInput").ap()
        for f in range(1, FO)
    }
    xrests = {}
    if Cs[0] > c00:
        xrests["xr0", c00, Cs[0]] = nc.dram_tensor(
            "xr0", [P, DO, Cs[0] - c00], BF16, kind="ExternalInput"
        ).ap()
    for g in range(1, NUM_GROUPS):
        xrests[f"xg{g}", offs[g], offs[g] + Cs[g]] = nc.dram_tensor(
            f"xg{g}", [P, DO, Cs[g]], BF16, kind="ExternalInput"
        ).ap()
    bgus = {
        g: nc.dram_tensor(f"bgu{g}", [P, DO, 2 * FQ], BF16, kind="ExternalInput").ap()
        for g in range(1, NUM_GROUPS)
    }
    bdws = [
        nc.dram_tensor(f"bdw{g}", [P, FO, D_MODEL], BF16, kind="ExternalInput").ap()
        for g in range(NUM_GROUPS)
    ]
    # per-(group, chunk) contiguous output blocks [P, DO, c] fp16
    ybs = {
        (g, cc): nc.dram_tensor(
            f"yb{g}_{cc}", [P, DO, c], F16, kind="ExternalOutput"
        ).ap()
        for g in range(NUM_GROUPS)
        for cc, c in enumerate(caps[g])
    }
    wy = nc.dram_tensor("wy", [P, 64], F32, kind="ExternalOutput").ap()

    with tile.TileContext(nc) as tc:
        with (
            tc.tile_pool(name="wu", bufs=1) as wup,
            tc.tile_pool(name="xp", bufs=1) as xp,
            tc.tile_pool(name="hp", bufs=1) as hp,
            tc.tile_pool(name="gp", bufs=1) as gp,
            tc.tile_pool(name="dp", bufs=1) as dp,
            tc.tile_pool(name="sp", bufs=4) as sp,
            tc.tile_pool(name="yp", bufs=2) as yp,
            tc.tile_pool(name="pg", bufs=2, space="PSUM") as pgp,
            tc.tile_pool(name="pu", bufs=2, space="PSUM") as pup,
            tc.tile_pool(name="pd", bufs=4, space="PSUM") as pdp,
        ):
            # boot DMA first: first x chunk + fo0 gate/up weights in one
            # contiguous transfer on the sync HWDGE queue.
            bt1 = xp.tile([P, DO, c00 + 2 * P], BF16, tag="bt1")
            nc.sync.dma_start(bt1[:], boot1[:])

            # ── PE warm-up: dense dummy matmuls while the startup DMAs fly ──
            wt = wup.tile([P, P], BF16, tag="wt")
            nc.vector.memset(wt[:], 0.0)
            pws = pgp.tile([P, 64], F32, tag="psg", name="psg_warm")
            for i in range(N_WARM):
                nc.tensor.matmul(
                    pws[:], wt[:], wt[:, 0:64],
                    start=(i == 0), stop=(i == N_WARM - 1),
                )

            # ── remaining input DMAs on ONE queue (sync), in need-order ─────
            # HBM bandwidth (~440 GB/s/core) is shared across queues, so a
            # second concurrent input stream only delays the critical path.
            wfts = {}
            for f in range(1, FO):
                wfts[f] = gp.tile([P, DO, 2 * P], BF16, tag=f"wf{f}", name=f"wf_{f}")
                nc.sync.dma_start(wfts[f][:], wfos[f][:])
            xss = xp.tile([P, DO, W], BF16, tag="xs")
            for (nm, a, b), src in xrests.items():
                if nm != "xr0":
                    continue
                nc.sync.dma_start(xss[:, :, a:b], src[:])
            guts = {}
            for g in range(1, NUM_GROUPS):
                guts[g] = gp.tile(
                    [P, DO, 2 * FQ], BF16, tag=f"gut{g}", name=f"gut_{g}"
                )
                nc.sync.dma_start(guts[g][:], bgus[g][:])
                for (nm, a, b), src in xrests.items():
                    if nm != f"xg{g}":
                        continue
                    nc.sync.dma_start(xss[:, :, a:b], src[:])
            dts = {}
            for g in range(NUM_GROUPS):
                dts[g] = dp.tile([P, FO, D_MODEL], BF16, tag=f"dt{g}", name=f"dt_{g}")
                nc.sync.dma_start(dts[g][:], bdws[g][:])

            # keep the warm-up matmuls from being dead-code-eliminated
            wys = wup.tile([P, 64], F32, tag="wy")
            nc.vector.tensor_copy(out=wys[:], in_=pws[:])
            nc.sync.dma_start(wy[:, :], wys[:])

            # ── phase 1: gate/up + SwiGLU for all 4 group segments ──────────
            hs = {}
            for g in range(NUM_GROUPS):
                hs[g] = hp.tile([P, FO, Cs[g]], BF16, tag=f"h{g}", name=f"h{g}")
                coff = 0
                for cc, chunk in enumerate(caps[g]):
                    cs = slice(coff, coff + chunk)
                    xcs = slice(offs[g] + coff, offs[g] + coff + chunk)
                    for fo in range(FO):
                        if g == 0:
                            if fo == 0:
                                gt_, go_, ut_, uo_ = bt1, c00, bt1, c00 + P
                            else:
                                gt_, go_, ut_, uo_ = wfts[fo], 0, wfts[fo], P
                        else:
                            gt_, go_ = guts[g], fo * P
                            ut_, uo_ = guts[g], FQ + fo * P
                        psg = pgp.tile([P, chunk], F32, tag="psg",
                                       name=f"psg_{g}_{cc}_{fo}")
                        psu = pup.tile([P, chunk], F32, tag="psu",
                                       name=f"psu_{g}_{cc}_{fo}")
                        boot = g == 0 and cc == 0
                        for do in range(DO):
                            nc.tensor.matmul(
                                psg[:], gt_[:, do, go_ : go_ + P],
                                bt1[:, do, 0:c00] if boot else xss[:, do, xcs],
                                start=(do == 0), stop=(do == DO - 1),
                            )
                        for do in range(DO):
                            nc.tensor.matmul(
                                psu[:], ut_[:, do, uo_ : uo_ + P],
                                bt1[:, do, 0:c00] if boot else xss[:, do, xcs],
                                start=(do == 0), stop=(do == DO - 1),
                            )
                        sg = sp.tile([P, chunk], F32, tag="sg")
                        nc.scalar.activation(
                            sg[:], psg[:], mybir.ActivationFunctionType.Silu
                        )
                        nc.vector.tensor_mul(
                            out=hs[g][:, fo, cs], in0=sg[:], in1=psu[:]
                        )
                    coff += chunk

            # ── phase 2: down-projection for all 4 group segments ───────────
            # outputs staged per (g, cc) in one [P, DO, c] tile, shipped as a
            # single fully-contiguous DMA -> large packets, tiny tail.
            nq = 0
            for g in range(NUM_GROUPS):
                coff = 0
                for cc, chunk in enumerate(caps[g]):
                    cs = slice(coff, coff + chunk)
                    yo = yp.tile([P, DO, chunk], F16, tag="yo",
                                 name=f"yo_{g}_{cc}")
                    for do in range(DO):
                        psy = pdp.tile([P, chunk], F32, tag="psy",
                                       name=f"psy_{g}_{cc}_{do}")
                        for fo in range(FO):
                            nc.tensor.matmul(
                                psy[:],
                                dts[g][:, fo, do * P : (do + 1) * P],
                                hs[g][:, fo, cs],
                                start=(fo == 0), stop=(fo == FO - 1),
                            )
                        if nq % 2 == 0:
                            nc.scalar.activation(
                                yo[:, do, :], psy[:],
                                mybir.ActivationFunctionType.Copy,
                            )
                        else:
                            nc.vector.tensor_copy(out=yo[:, do, :], in_=psy[:])
                        nq += 1
                        # ship each half as soon as its 4 do-slices are
                        # copied: the trailing DMA after the last matmul is
                        # only half a block (~0.3 MB) instead of 1+ MB.  The
                        # very last block ships in quarters so the final
                        # piece is ~0.14 MB.
                        last_blk = (g == NUM_GROUPS - 1
                                    and cc == len(caps[g]) - 1)
                        if do == 3:
                            nc.sync.dma_start(
                                ybs[g, cc][:, 0:4, :], yo[:, 0:4, :]
                            )
                        elif last_blk and do == 5:
                            nc.sync.dma_start(
                                ybs[g, cc][:, 4:6, :], yo[:, 4:6, :]
                            )
                        elif last_blk and do == DO - 1:
                            nc.sync.dma_start(
                                ybs[g, cc][:, 6:DO, :], yo[:, 6:DO, :]
                            )
                        elif do == DO - 1:
                            nc.sync.dma_start(
                                ybs[g, cc][:, 4:DO, :], yo[:, 4:DO, :]
                            )
                    coff += chunk
    nc.finalize()
    return nc


def _get_program(caps):
    if caps not in _BUILD_CACHE:
        _BUILD_CACHE[caps] = _build(caps)
    return _BUILD_CACHE[caps]


def _sigmoid(z):
    return 1.0 / (1.0 + np.exp(-z))


def _route(xf32, macro_w, micro_w):
    """Host routers in float64. Returns group index per token and per-token
    weights for the 2 experts of the selected group (float32)."""
    xf = xf32.astype(np.float64)
    ms = _sigmoid(xf @ macro_w.astype(np.float64))  # [T, G]
    g_sel = np.argmax(ms, axis=1)
    T = xf.shape[0]
    mval = ms[np.arange(T), g_sel]
    mv = mval / (mval + EPS)

    w2 = np.zeros((T, 2), np.float64)
    for g in range(NUM_GROUPS):
        idx = np.nonzero(g_sel == g)[0]
        if idx.size == 0:
            continue
        s = _sigmoid(xf[idx] @ micro_w[g].astype(np.float64))  # [n, 2]
        denom = np.maximum(s[:, 0], s[:, 1]) + np.minimum(s[:, 0], s[:, 1]) + EPS
        w2[idx, 0] = mv[idx] * s[:, 0] / denom
        w2[idx, 1] = mv[idx] * s[:, 1] / denom
    return g_sel, w2.astype(np.float32)


def _pad4(v: int) -> int:
    return -(-v // 4) * 4


def _even_chunks(n: int, cmax: int = 512) -> list[int]:
    """Even split of n into ceil(n/cmax) chunks, each padded to %4."""
    n = max(n, 8)
    k = -(-n // cmax)
    base = -(-n // k)
    out = [_pad4(base)] * (k - 1)
    out.append(_pad4(n - base * (k - 1)))
    return out


def _plan_chunks(counts: list[int]) -> tuple[tuple[int, ...], ...]:
    """Chunk plan per group: small boot chunk for group 0 (fast first matmul),
    small final chunk for the last group (tiny trailing output DMA)."""
    plans = []
    for g, n in enumerate(counts):
        n = max(n, 8)
        if g == 0:
            if n <= 288:
                plans.append((_pad4(n),))
            else:
                plans.append((256, *_even_chunks(n - 256)))
        else:
            plans.append(tuple(_even_chunks(n)))
    return tuple(plans)


def kernel(x, macro_w, micro_w, gate_w, up_w, down_w):
    global LAST_RESULTS
    x = np.asarray(x)
    B, S, D = x.shape
    T = B * S
    xf = np.ascontiguousarray(x.reshape(T, D).astype(np.float32, copy=False))

    g_sel, w2 = _route(xf, np.asarray(macro_w), np.asarray(micro_w))
    idx_by_g = [np.nonzero(g_sel == g)[0] for g in range(NUM_GROUPS)]

    caps = _plan_chunks([ix.size for ix in idx_by_g])
    Cs = [sum(ch) for ch in caps]
    offs = [sum(Cs[:g]) for g in range(NUM_GROUPS)]
    W = sum(Cs)
    c00 = caps[0][0]
    nc = _get_program(caps)

    # group-sorted, padded token matrix [D, W] bf16 (replicated to all cores)
    xs = np.zeros((D, W), ml_dtypes.bfloat16)
    for g in range(NUM_GROUPS):
        ix = idx_by_g[g]
        if ix.size:
            xs[:, offs[g] : offs[g] + ix.size] = xf[ix].T.astype(ml_dtypes.bfloat16)

    # bf16 weights in partition-major [p, do/fo, f/d] layout (contiguous DMA)
    gate_b = np.asarray(gate_w, np.float32).astype(ml_dtypes.bfloat16)
    up_b = np.asarray(up_w, np.float32).astype(ml_dtypes.bfloat16)
    down_b = np.asarray(down_w, np.float32).astype(ml_dtypes.bfloat16)
    # [E, D, F] -> [E, DO, P, F] -> [E, P, DO, F]
    gate_p = gate_b.reshape(NUM_EXPERTS, DO, P, FFN_DIM).transpose(0, 2, 1, 3)
    up_p = up_b.reshape(NUM_EXPERTS, DO, P, FFN_DIM).transpose(0, 2, 1, 3)
    # [E, F, D] -> [E, 4, FO, P, D] -> [E, 4, P, FO, D]
    down_p = down_b.reshape(NUM_EXPERTS, 4, FO, P, D_MODEL).transpose(0, 1, 3, 2, 4)

    # partition-major token array [p, do, c]; ship as contiguous blocks
    xsp = xs.reshape(DO, P, W).transpose(1, 0, 2)
    xboot = np.ascontiguousarray(xsp[:, :, 0:c00])
    xparts = {}
    if Cs[0] > c00:
        xparts["xr0"] = np.ascontiguousarray(xsp[:, :, c00 : Cs[0]])
    for g in range(1, NUM_GROUPS):
        xparts[f"xg{g}"] = np.ascontiguousarray(
            xsp[:, :, offs[g] : offs[g] + Cs[g]]
        )

    in_maps = []
    for c in range(N_CORES):
        m = dict(xparts)
        b = c // 4  # which expert of each group
        q = c % 4  # which F-quarter
        fsl = slice(q * FQ, (q + 1) * FQ)
        for g in range(NUM_GROUPS):
            e = 2 * g + b
            if g == 0:
                gq = gate_p[e][:, :, fsl]
                uq = up_p[e][:, :, fsl]
                m["boot1"] = np.ascontiguousarray(
                    np.concatenate(
                        [xboot, gq[:, :, 0:P], uq[:, :, 0:P]], axis=2
                    )
                )
                for f in range(1, FO):
                    m[f"wfo{f}"] = np.ascontiguousarray(
                        np.concatenate(
                            [
                                gq[:, :, f * P : (f + 1) * P],
                                uq[:, :, f * P : (f + 1) * P],
                            ],
                            axis=2,
                        )
                    )
            else:
                bgu = np.empty((P, DO, 2 * FQ), ml_dtypes.bfloat16)
                bgu[:, :, :FQ] = gate_p[e][:, :, fsl]
                bgu[:, :, FQ:] = up_p[e][:, :, fsl]
                m[f"bgu{g}"] = bgu
            m[f"bdw{g}"] = np.ascontiguousarray(down_p[e, q])
        in_maps.append(m)

    res = run_bass_kernel_spmd(nc, in_maps, core_ids=list(range(N_CORES)))
    LAST_RESULTS = res

    y = np.zeros((T, D), np.float32)
    for g in range(NUM_GROUPS):
        ix = idx_by_g[g]
        if ix.size == 0:
            continue
        # stitch the per-chunk blocks into [D, C_g] partials per expert half
        pa = np.zeros((D, Cs[g]), np.float32)
        pb = np.zeros((D, Cs[g]), np.float32)
        coff = 0
        for cc, chunk in enumerate(caps[g]):
            for c in range(N_CORES):
                blk = res.results[c][f"yb{g}_{cc}"].astype(np.float32)
                # blk[p, do, j] = y_d[do*128+p] for token j
                part = blk.transpose(1, 0, 2).reshape(D, chunk)
                if c < 4:
                    pa[:, coff : coff + chunk] += part
                else:
                    pb[:, coff : coff + chunk] += part
            coff += chunk
        seg = slice(0, ix.size)
        y[ix] = pa[:, seg].T * w2[ix, 0:1] + pb[:, seg].T * w2[ix, 1:2]
    return y.reshape(B, S, D)
